# revision 36
# baseline (speedup 1.0000x reference)
"""Causal self-attention (B=4, T=2048, D=1024, H=16) on 8 Trainium2 cores.

Sharding: core c handles batch b = c // 2 and head-half = c % 2 (8 of the 16
heads). Zero cross-core communication: each core computes q/k/v projections
for its 8 heads, causal flash-style attention, and a partial output
projection against its half of w_o. The host sums the two partial
projections per batch.

Layouts (every matmul operand is a direct slice, no on-device transposes):
  xT    (1024, 2048)  x[b].T            stationary slices for q/k/v
  wqT   (1024, 512)   w_q[rows].T       (softmax scale applied in the exp)
  wkT   (1024, 512)   w_k[rows].T
  wvT   (1024, 512)   w_v[rows].T
  woT   (512, 1024)   w_o[:, cols].T
  poutT (1024, 2048)  partial (x @ w_o.T contribution).T

Attention per head (dh=64): scores computed TRANSPOSED (k on partitions,
q on free dim) so softmax tiles feed P@V directly as the moving operand.
The two heads of a head-pair run as row-tiled concurrent matmuls
(tile_position (0,0)/(64,0)).  exp runs on the scalar engine with
scale=1/8 folded in; the denominator comes from a 65th all-ones column
appended to v; normalization = DVE reciprocal + GPSIMD partition
broadcast + DVE multiply.

Causal structure: for q-block qb (512 wide) the k tiles 4qb..4qb+3 are
"diagonal": columns left of the triangle window are fully masked, so
scores / exp / P@V all skip them, and the GPSIMD affine_select mask only
touches the 128-wide triangle window.

Scheduling: one flat issue order.  The attention inner loops are
ACT(exp)-bound, so q/k projections for the NEXT head-pair and output
projection tiles are issued as small "filler" bursts inside the
attention slots to keep the tensor engine dense (HAM stays warm).

Everything 16-bit (fp16) into the PE; PSUM and the final partial output
stay fp32.
"""
import sys

if "/opt/trn_rl_repo" not in sys.path:
    sys.path.insert(0, "/opt/trn_rl_repo")

import numpy as np

B, T, D, H = 4, 2048, 1024, 16
P, TQ = 128, 512
ND = D // P          # 8  d-slices (contraction tiles for projections)
NHP = 4              # head-pairs per core (8 heads)
NQB = T // TQ        # 4  q blocks
NKB = T // P         # 16 k tiles

PRECISION = "fp16"

_COMPILED = {}


def _build(precision=None):
    import concourse.bacc as bacc
    import concourse.tile as tile
    from concourse import mybir
    from contextlib import ExitStack

    F32 = mybir.dt.float32
    F32R = mybir.dt.float32r
    F16 = mybir.dt.float16
    AF = mybir.ActivationFunctionType

    nc = bacc.Bacc("TRN2", target_bir_lowering=False, debug=False, num_devices=8)

    xT = nc.dram_tensor("xT", [D, T], F16, kind="ExternalInput")
    wqT = nc.dram_tensor("wqT", [D, 512], F16, kind="ExternalInput")
    wkT = nc.dram_tensor("wkT", [D, 512], F16, kind="ExternalInput")
    wvT = nc.dram_tensor("wvT", [D, 512], F16, kind="ExternalInput")
    woT = nc.dram_tensor("woT", [512, D], F16, kind="ExternalInput")
    pout = nc.dram_tensor("poutT", [D, T], F32, kind="ExternalOutput")

    with tile.TileContext(nc) as tc:
        with ExitStack() as ctx:
            xt_pool = ctx.enter_context(tc.tile_pool(name="xt", bufs=ND))
            w_pool = ctx.enter_context(tc.tile_pool(name="w", bufs=3 * ND + 1))
            wo_pool = ctx.enter_context(tc.tile_pool(name="wo", bufs=4))
            v_pool = ctx.enter_context(tc.tile_pool(name="v", bufs=NKB))
            q_pool = ctx.enter_context(tc.tile_pool(name="q", bufs=NHP))
            k_pool = ctx.enter_context(tc.tile_pool(name="k", bufs=NHP))
            ao_pool = ctx.enter_context(tc.tile_pool(name="ao", bufs=NHP))
            p_pool = ctx.enter_context(tc.tile_pool(name="p", bufs=4))
            r_pool = ctx.enter_context(tc.tile_pool(name="r", bufs=8))
            po_pool = ctx.enter_context(tc.tile_pool(name="po", bufs=4))
            mm_psum = ctx.enter_context(
                tc.tile_pool(name="mmps", bufs=2, space="PSUM"))
            s_psum = ctx.enter_context(
                tc.tile_pool(name="sps", bufs=2, space="PSUM"))
            o_psum = ctx.enter_context(
                tc.tile_pool(name="ops", bufs=2, space="PSUM"))

            xt = [xt_pool.tile([P, T], F16, tag="xt", name="xt")
                  for _ in range(ND)]
            wqs = [w_pool.tile([P, 512], F16, tag="w", name="wq")
                   for _ in range(ND)]
            wks = [w_pool.tile([P, 512], F16, tag="w", name="wk")
                   for _ in range(ND)]
            wvs = [w_pool.tile([P, 512], F16, tag="w", name="wv")
                   for _ in range(ND)]
            wos = [wo_pool.tile([P, D], F16, tag="wo", name="wo")
                   for _ in range(4)]
            vA = [v_pool.tile([P, 8, 65], F16, tag="vA", name="vA")
                  for _ in range(NKB)]
            qT = [q_pool.tile([P, T], F16, tag="qT", name="qT")
                  for _ in range(NHP)]
            kT = [k_pool.tile([P, T], F16, tag="kT", name="kT")
                  for _ in range(NHP)]
            aoT = [ao_pool.tile([P, T], F16, tag="aoT", name="aoT")
                   for _ in range(NHP)]
            ones_col = w_pool.tile([P, 8, 1], F16, tag="ones", name="ones")

            # ---------------- input DMA ----------------
            # x arrives in 512-column chunks so v-projection can start after
            # ~1/4 of x is resident; v weights ride along with the first
            # chunk, q/k/o weights after the rest of x.
            for ds in range(ND):
                nc.sync.dma_start(wvs[ds], wvT[ds * P:(ds + 1) * P, :])
                nc.sync.dma_start(xt[ds][:, 0:TQ],
                                  xT[ds * P:(ds + 1) * P, 0:TQ])
            for cc in range(1, 4):
                for ds in range(ND):
                    nc.sync.dma_start(
                        xt[ds][:, cc * TQ:(cc + 1) * TQ],
                        xT[ds * P:(ds + 1) * P, cc * TQ:(cc + 1) * TQ])
                if cc < 3:
                    for w_dram, wts in ((wqT, wqs), (wkT, wks)):
                        for ds in range(4 * (cc - 1), 4 * cc):
                            nc.sync.dma_start(
                                wts[ds], w_dram[ds * P:(ds + 1) * P, :])
            for cs in range(4):
                nc.sync.dma_start(wos[cs], woT[cs * P:(cs + 1) * P, :])
            nc.vector.memset(ones_col[:], 1.0)
            ones1 = r_pool.tile([1, 64], F32R, tag="ones1", name="ones1")
            ones1f = r_pool.tile([1, 64], F32, tag="ones1f", name="ones1f")
            nc.vector.memset(ones1f[:], 1.0)
            nc.vector.tensor_copy(ones1[:], ones1f[:])

            # ---------------- projection unit machinery ----------------
            # v-chunk: two k-position tiles, each accumulating 8
            # contraction steps in a PSUM tile (16 matmuls, atomic).
            def v_chunk(kc):
                def go():
                    ps = [mm_psum.tile([P, TQ], F32, tag="mm", name="vmm")
                          for _ in range(2)]
                    for ds in range(ND):
                        for i in range(2):
                            kb = 2 * kc + i
                            nc.tensor.matmul(
                                ps[i],
                                xt[ds][:, kb * P:(kb + 1) * P],
                                wvs[ds][:],
                                start=(ds == 0), stop=(ds == ND - 1))
                    for i in range(2):
                        kb = 2 * kc + i
                        nc.vector.tensor_copy(
                            vA[kb][:, :, 0:64],
                            ps[i][:].rearrange("p (h c) -> p h c", c=64))
                        nc.vector.tensor_copy(
                            vA[kb][:, :, 64:65], ones_col[:])
                return go

            # q/k unit: one (w, hp, tt) output tile, split in two 4-matmul
            # halves so it can be spread across two fill points.  At most
            # one other mm-pool allocation may occur between the halves
            # (the 2-buffer rotation then stays clear of the held tile);
            # the norm below was shaped to respect that.
            def qk_halves(wts, dst, hp, tt):
                cell = {}

                def first():
                    cell["ps"] = mm_psum.tile([P, TQ], F32, tag="mm",
                                              name="qkmm")
                    for ds in range(4):
                        nc.tensor.matmul(
                            cell["ps"],
                            wts[ds][:, hp * P:(hp + 1) * P],
                            xt[ds][:, tt * TQ:(tt + 1) * TQ],
                            start=(ds == 0), stop=False)

                def second():
                    ps = cell.pop("ps")
                    for ds in range(4, ND):
                        nc.tensor.matmul(
                            ps,
                            wts[ds][:, hp * P:(hp + 1) * P],
                            xt[ds][:, tt * TQ:(tt + 1) * TQ],
                            start=False, stop=(ds == ND - 1))
                    nc.vector.tensor_copy(
                        dst[:, tt * TQ:(tt + 1) * TQ], ps[:])

                return [first, second]

            def outproj_group(od, tt):
                def go():
                    ps = mm_psum.tile([P, TQ], F32, tag="mm", name="pomm")
                    for cs in range(4):
                        nc.tensor.matmul(
                            ps,
                            wos[cs][:, od * P:(od + 1) * P],
                            aoT[cs][:, tt * TQ:(tt + 1) * TQ],
                            start=(cs == 0), stop=(cs == 3))
                    po = po_pool.tile([P, TQ], F32, tag="po", name="po")
                    nc.vector.tensor_copy(po[:], ps[:])
                    nc.sync.dma_start(
                        pout[od * P:(od + 1) * P, tt * TQ:(tt + 1) * TQ],
                        po[:])
                return go

            # filler: deadline-tagged PE bursts issued inside attention
            # slots.  deadline = first global slot index (hp*4+qb) whose
            # attention requires the unit's output.  When the projection
            # supply runs out, fall through to out-projection groups.
            filler = []
            for hp in range(NHP):
                for tt in range(NQB):
                    if hp == 0 and tt < 2:
                        continue  # issued upfront below
                    for h in qk_halves(wqs, qT[hp], hp, tt):
                        filler.append((4 * hp + tt, h))
                    for h in qk_halves(wks, kT[hp], hp, tt):
                        filler.append((4 * hp + tt, h))
            filler.sort(key=lambda e: e[0])
            fill_pos = [0]
            out_groups = []
            out_pos = [0]

            def consume_filler(n):
                for _ in range(n):
                    if fill_pos[0] < len(filler):
                        filler[fill_pos[0]][1]()
                        fill_pos[0] += 1
                    elif out_pos[0] < len(out_groups):
                        out_groups[out_pos[0]]()
                        out_pos[0] += 1
                    else:
                        break

            def consume_due(slot_idx):
                while fill_pos[0] < len(filler) and \
                        filler[fill_pos[0]][0] <= slot_idx:
                    filler[fill_pos[0]][1]()
                    fill_pos[0] += 1

            def consume_outproj(n):
                a = out_pos[0]
                b = min(a + n, len(out_groups))
                for i in range(a, b):
                    out_groups[i]()
                out_pos[0] = b

            # upfront: all of v, and q/k for (hp=0, tt=0..1); later q/k
            # tiles stream in as attention-slot filler.
            for kc in range(NKB // 2):
                v_chunk(kc)()
            for tt in range(2):
                for unit in (qk_halves(wqs, qT[0], 0, tt),
                             qk_halves(wks, kT[0], 0, tt)):
                    for h in unit:
                        h()

            # ---------------- attention ----------------
            # each slot's normalization is deferred into the next slot so
            # the ~3us DVE->PE->DVE chain runs while the PE streams the next
            # slot's scores; o_psum rotation (bufs=2) then unblocks in time.
            pending_norm = [None]

            def attention_slot(hp, qb):
                consume_due(4 * hp + qb)
                diag = [4 * qb + i for i in range(4)]
                order = diag + list(range(4 * qb))
                n_kb = len(order)
                o_ps = [o_psum.tile([P, TQ], F32, tag="o", name="o")
                        for _ in range(2)]
                s_tiles = {}
                pts = {}

                def c0_of(kb):
                    return max(0, kb * P - qb * TQ)

                def issue_scores(kb):
                    c0 = c0_of(kb)
                    sp = s_psum.tile([P, 2, TQ], F32, tag="s", name="s")
                    for j in range(2):
                        nc.tensor.matmul(
                            sp[:, j, c0:TQ],
                            kT[hp][j * 64:(j + 1) * 64, kb * P:(kb + 1) * P],
                            qT[hp][j * 64:(j + 1) * 64,
                                   qb * TQ + c0:(qb + 1) * TQ],
                            tile_position=(j * 64, 0))
                    s_tiles[kb] = (sp, c0)

                def issue_exp(kb):
                    sp, c0 = s_tiles.pop(kb)
                    pt = p_pool.tile([P, 2, TQ], F16, tag="p", name="p")
                    nc.scalar.activation(pt[:, :, c0:TQ], sp[:, :, c0:TQ],
                                         AF.Exp, scale=0.125)
                    if kb >= 4 * qb:
                        # triangle window of the diagonal tile
                        nc.gpsimd.affine_select(
                            out=pt[:, :, c0:c0 + P], in_=pt[:, :, c0:c0 + P],
                            pattern=[[0, 2], [1, P]],
                            compare_op=mybir.AluOpType.is_ge,
                            fill=0.0, base=0, channel_multiplier=-1)
                    pts[kb] = (pt, c0)

                def issue_pv(kb, first, last):
                    pt, c0 = pts.pop(kb)
                    for j in range(2):
                        nc.tensor.matmul(
                            o_ps[j][0:65, c0:TQ],
                            vA[kb][:, 2 * hp + j, :],
                            pt[:, j, c0:TQ],
                            start=first, stop=last)

                issue_scores(order[0])
                issue_scores(order[1])
                if pending_norm[0] is not None:
                    consume_filler(1)
                    pending_norm[0]()
                    pending_norm[0] = None
                issue_exp(order[0])
                consume_filler(1)
                for i, kb in enumerate(order):
                    if i + 2 < n_kb:
                        issue_scores(order[i + 2])
                    if i + 1 < n_kb:
                        issue_exp(order[i + 1])
                    issue_pv(kb, first=(i == 0), last=(i == n_kb - 1))
                    if (qb == 2 and i % 3 == 2) or (qb == 3 and i % 2 == 1):
                        consume_filler(1)

                # normalization: rows 0..63 of o_ps = unnormalized out.T,
                # row 64 = sum(exp).  Sums copied out, then their
                # reciprocal is broadcast across 64 partitions via a K=1
                # ones matmul into the unused upper half of the o_ps bank,
                # and the multiply writes aoT.
                def norm():
                    for j in range(2):
                        sc = r_pool.tile([1, TQ], F32R, tag="sc", name="sc")
                        nc.vector.tensor_copy(sc[:], o_ps[j][64:65, :])
                        rb = mm_psum.tile([P, TQ], F32, tag="mm", name="rb")
                        nc.tensor.matmul(rb[0:64, :], ones1[:], sc[:],
                                         start=True, stop=True)
                        R = r_pool.tile([64, TQ], F32, tag="Rb", name="R")
                        nc.vector.reciprocal_approx_fast(R[:], rb[0:64, :])
                        nc.vector.tensor_mul(
                            aoT[hp][j * 64:(j + 1) * 64,
                                    qb * TQ:(qb + 1) * TQ],
                            o_ps[j][0:64, :], R[:])
                pending_norm[0] = norm

            # out-projection groups for tt become available once hp=3's
            # norm for qb=tt has been issued (inside the NEXT slot, since
            # norms are deferred).
            for hp in range(NHP):
                for qb in range(NQB):
                    attention_slot(hp, qb)
                    if hp == 3 and qb >= 1:
                        out_groups.extend(
                            outproj_group(od, qb - 1) for od in range(ND))
                        consume_outproj(8)
            pending_norm[0]()
            pending_norm[0] = None
            out_groups.extend(outproj_group(od, 3) for od in range(ND))
            consume_filler(len(filler) - fill_pos[0])
            consume_outproj(len(out_groups))

    nc.compile()
    return nc


def _get_compiled(precision=None):
    key = precision or PRECISION
    if key not in _COMPILED:
        _COMPILED[key] = _build(key)
    return _COMPILED[key]


def make_in_maps(x, w_q, w_k, w_v, w_o, precision=None):
    xTs = [np.ascontiguousarray(x[b].T).astype(np.float16) for b in range(B)]
    wq = [np.ascontiguousarray(w_q[h * 512:(h + 1) * 512].T).astype(np.float16)
          for h in range(2)]
    wk = [np.ascontiguousarray(w_k[h * 512:(h + 1) * 512].T).astype(np.float16)
          for h in range(2)]
    wv = [np.ascontiguousarray(w_v[h * 512:(h + 1) * 512].T).astype(np.float16)
          for h in range(2)]
    wo = [np.ascontiguousarray(w_o[:, h * 512:(h + 1) * 512].T).astype(np.float16)
          for h in range(2)]
    in_maps = []
    for c in range(8):
        b, half = divmod(c, 2)
        in_maps.append({
            "xT": xTs[b],
            "wqT": wq[half],
            "wkT": wk[half],
            "wvT": wv[half],
            "woT": wo[half],
        })
    return in_maps


def kernel(x, w_q, w_k, w_v, w_o):
    from concourse.bass_utils import run_bass_kernel_spmd

    x = np.asarray(x, dtype=np.float32)
    w_q = np.asarray(w_q, dtype=np.float32)
    w_k = np.asarray(w_k, dtype=np.float32)
    w_v = np.asarray(w_v, dtype=np.float32)
    w_o = np.asarray(w_o, dtype=np.float32)

    nc = _get_compiled()
    in_maps = make_in_maps(x, w_q, w_k, w_v, w_o)
    res = run_bass_kernel_spmd(nc, in_maps, list(range(8)))

    out = np.empty((B, T, D), dtype=np.float32)
    for b in range(B):
        out[b] = (res.results[2 * b]["poutT"] + res.results[2 * b + 1]["poutT"]).T
    return out


# revision 37
# speedup vs baseline: 1.0374x; 1.0374x over previous
"""Causal self-attention (B=4, T=2048, D=1024, H=16) on 8 Trainium2 cores.

Sharding: core c handles batch b = c // 2 and head-half = c % 2 (8 of the 16
heads). Zero cross-core communication: each core computes q/k/v projections
for its 8 heads, causal flash-style attention, and a partial output
projection against its half of w_o. The host sums the two partial
projections per batch.

Layouts (every matmul operand is a direct slice, no on-device transposes):
  xT    (1024, 2048)  x[b].T            stationary slices for q/k/v
  wqT   (1024, 512)   w_q[rows].T       (softmax scale applied in the exp)
  wkT   (1024, 512)   w_k[rows].T
  wvT   (1024, 512)   w_v[rows].T
  woT   (512, 1024)   w_o[:, cols].T
  poutT (1024, 2048)  partial (x @ w_o.T contribution).T

Attention per head (dh=64): scores computed TRANSPOSED (k on partitions,
q on free dim) so softmax tiles feed P@V directly as the moving operand.
The two heads of a head-pair run as row-tiled concurrent matmuls
(tile_position (0,0)/(64,0)).  exp runs on the scalar engine with
scale=1/8 folded in; the denominator comes from a 65th all-ones column
appended to v; normalization = DVE reciprocal + GPSIMD partition
broadcast + DVE multiply.

Causal structure: for q-block qb (512 wide) the k tiles 4qb..4qb+3 are
"diagonal": columns left of the triangle window are fully masked, so
scores / exp / P@V all skip them, and the GPSIMD affine_select mask only
touches the 128-wide triangle window.

Scheduling: one flat issue order.  The attention inner loops are
ACT(exp)-bound, so q/k projections for the NEXT head-pair and output
projection tiles are issued as small "filler" bursts inside the
attention slots to keep the tensor engine dense (HAM stays warm).

Everything 16-bit (fp16) into the PE; PSUM and the final partial output
stay fp32.
"""
import sys

if "/opt/trn_rl_repo" not in sys.path:
    sys.path.insert(0, "/opt/trn_rl_repo")

import numpy as np

B, T, D, H = 4, 2048, 1024, 16
P, TQ = 128, 512
ND = D // P          # 8  d-slices (contraction tiles for projections)
NHP = 4              # head-pairs per core (8 heads)
NQB = T // TQ        # 4  q blocks
NKB = T // P         # 16 k tiles

PRECISION = "fp16"

_COMPILED = {}


def _build(precision=None):
    import concourse.bacc as bacc
    import concourse.tile as tile
    from concourse import mybir
    from contextlib import ExitStack

    F32 = mybir.dt.float32
    F32R = mybir.dt.float32r
    F16 = mybir.dt.float16
    AF = mybir.ActivationFunctionType

    nc = bacc.Bacc("TRN2", target_bir_lowering=False, debug=False, num_devices=8)

    xT = nc.dram_tensor("xT", [D, T], F16, kind="ExternalInput")
    wqT = nc.dram_tensor("wqT", [D, 512], F16, kind="ExternalInput")
    wkT = nc.dram_tensor("wkT", [D, 512], F16, kind="ExternalInput")
    wvT = nc.dram_tensor("wvT", [D, 512], F16, kind="ExternalInput")
    woT = nc.dram_tensor("woT", [512, D], F16, kind="ExternalInput")
    pout = nc.dram_tensor("poutT", [D, T], F32, kind="ExternalOutput")

    with tile.TileContext(nc) as tc:
        with ExitStack() as ctx:
            xt_pool = ctx.enter_context(tc.tile_pool(name="xt", bufs=ND))
            w_pool = ctx.enter_context(tc.tile_pool(name="w", bufs=3 * ND + 1))
            wo_pool = ctx.enter_context(tc.tile_pool(name="wo", bufs=4))
            v_pool = ctx.enter_context(tc.tile_pool(name="v", bufs=NKB))
            q_pool = ctx.enter_context(tc.tile_pool(name="q", bufs=NHP))
            k_pool = ctx.enter_context(tc.tile_pool(name="k", bufs=NHP))
            ao_pool = ctx.enter_context(tc.tile_pool(name="ao", bufs=NHP))
            p_pool = ctx.enter_context(tc.tile_pool(name="p", bufs=4))
            r_pool = ctx.enter_context(tc.tile_pool(name="r", bufs=8))
            po_pool = ctx.enter_context(tc.tile_pool(name="po", bufs=4))
            mm_psum = ctx.enter_context(
                tc.tile_pool(name="mmps", bufs=2, space="PSUM"))
            s_psum = ctx.enter_context(
                tc.tile_pool(name="sps", bufs=2, space="PSUM"))
            o_psum = ctx.enter_context(
                tc.tile_pool(name="ops", bufs=2, space="PSUM"))

            xt = [xt_pool.tile([P, T], F16, tag="xt", name="xt")
                  for _ in range(ND)]
            wqs = [w_pool.tile([P, 512], F16, tag="w", name="wq")
                   for _ in range(ND)]
            wks = [w_pool.tile([P, 512], F16, tag="w", name="wk")
                   for _ in range(ND)]
            wvs = [w_pool.tile([P, 512], F16, tag="w", name="wv")
                   for _ in range(ND)]
            wos = [wo_pool.tile([P, D], F16, tag="wo", name="wo")
                   for _ in range(4)]
            vA = [v_pool.tile([P, 8, 65], F16, tag="vA", name="vA")
                  for _ in range(NKB)]
            qT = [q_pool.tile([P, T], F16, tag="qT", name="qT")
                  for _ in range(NHP)]
            kT = [k_pool.tile([P, T], F16, tag="kT", name="kT")
                  for _ in range(NHP)]
            aoT = [ao_pool.tile([P, T], F16, tag="aoT", name="aoT")
                   for _ in range(NHP)]
            ones_col = w_pool.tile([P, 8, 1], F16, tag="ones", name="ones")

            # ---------------- input DMA ----------------
            # x arrives in 512-column chunks so v-projection can start after
            # ~1/4 of x is resident; v weights ride along with the first
            # chunk, q/k/o weights after the rest of x.
            for ds in range(ND):
                nc.sync.dma_start(wvs[ds], wvT[ds * P:(ds + 1) * P, :])
                nc.sync.dma_start(xt[ds][:, 0:TQ],
                                  xT[ds * P:(ds + 1) * P, 0:TQ])
            for cc in range(1, 4):
                for ds in range(ND):
                    nc.sync.dma_start(
                        xt[ds][:, cc * TQ:(cc + 1) * TQ],
                        xT[ds * P:(ds + 1) * P, cc * TQ:(cc + 1) * TQ])
                if cc < 3:
                    for w_dram, wts in ((wqT, wqs), (wkT, wks)):
                        for ds in range(4 * (cc - 1), 4 * cc):
                            nc.sync.dma_start(
                                wts[ds], w_dram[ds * P:(ds + 1) * P, :])
            for cs in range(4):
                nc.sync.dma_start(wos[cs], woT[cs * P:(cs + 1) * P, :])
            nc.vector.memset(ones_col[:], 1.0)
            ones1 = r_pool.tile([1, 64], F32R, tag="ones1", name="ones1")
            ones1f = r_pool.tile([1, 64], F32, tag="ones1f", name="ones1f")
            nc.vector.memset(ones1f[:], 1.0)
            nc.vector.tensor_copy(ones1[:], ones1f[:])

            # ---------------- projection unit machinery ----------------
            # v-chunk: two k-position tiles, each accumulating 8
            # contraction steps in a PSUM tile (16 matmuls, atomic).
            def v_chunk(kc):
                def go():
                    ps = [mm_psum.tile([P, TQ], F32, tag="mm", name="vmm")
                          for _ in range(2)]
                    for ds in range(ND):
                        for i in range(2):
                            kb = 2 * kc + i
                            nc.tensor.matmul(
                                ps[i],
                                xt[ds][:, kb * P:(kb + 1) * P],
                                wvs[ds][:],
                                start=(ds == 0), stop=(ds == ND - 1))
                    for i in range(2):
                        kb = 2 * kc + i
                        nc.vector.tensor_copy(
                            vA[kb][:, :, 0:64],
                            ps[i][:].rearrange("p (h c) -> p h c", c=64))
                        nc.vector.tensor_copy(
                            vA[kb][:, :, 64:65], ones_col[:])
                return go

            # q/k unit: one (w, hp, tt) output tile, split in two 4-matmul
            # halves so it can be spread across two fill points.  At most
            # one other mm-pool allocation may occur between the halves
            # (the 2-buffer rotation then stays clear of the held tile);
            # the norm below was shaped to respect that.
            def qk_halves(wts, dst, hp, tt):
                cell = {}

                def first():
                    cell["ps"] = mm_psum.tile([P, TQ], F32, tag="mm",
                                              name="qkmm")
                    for ds in range(4):
                        nc.tensor.matmul(
                            cell["ps"],
                            wts[ds][:, hp * P:(hp + 1) * P],
                            xt[ds][:, tt * TQ:(tt + 1) * TQ],
                            start=(ds == 0), stop=False)

                def second():
                    ps = cell.pop("ps")
                    for ds in range(4, ND):
                        nc.tensor.matmul(
                            ps,
                            wts[ds][:, hp * P:(hp + 1) * P],
                            xt[ds][:, tt * TQ:(tt + 1) * TQ],
                            start=False, stop=(ds == ND - 1))
                    nc.vector.tensor_copy(
                        dst[:, tt * TQ:(tt + 1) * TQ], ps[:])

                return [first, second]

            def outproj_group(od, tt):
                def go():
                    ps = mm_psum.tile([P, TQ], F32, tag="mm", name="pomm")
                    for cs in range(4):
                        nc.tensor.matmul(
                            ps,
                            wos[cs][:, od * P:(od + 1) * P],
                            aoT[cs][:, tt * TQ:(tt + 1) * TQ],
                            start=(cs == 0), stop=(cs == 3))
                    po = po_pool.tile([P, TQ], F32, tag="po", name="po")
                    nc.vector.tensor_copy(po[:], ps[:])
                    nc.sync.dma_start(
                        pout[od * P:(od + 1) * P, tt * TQ:(tt + 1) * TQ],
                        po[:])
                return go

            # filler: deadline-tagged PE bursts issued inside attention
            # slots.  deadline = first global slot index (hp*4+qb) whose
            # attention requires the unit's output.  When the projection
            # supply runs out, fall through to out-projection groups.
            filler = []
            for hp in range(NHP):
                for tt in range(NQB):
                    if hp == 0 and tt < 2:
                        continue  # issued upfront below
                    for h in qk_halves(wqs, qT[hp], hp, tt):
                        filler.append((4 * hp + tt, h))
                    for h in qk_halves(wks, kT[hp], hp, tt):
                        filler.append((4 * hp + tt, h))
            filler.sort(key=lambda e: e[0])
            fill_pos = [0]
            out_groups = []
            out_pos = [0]

            def consume_filler(n):
                for _ in range(n):
                    if fill_pos[0] < len(filler):
                        filler[fill_pos[0]][1]()
                        fill_pos[0] += 1
                    elif out_pos[0] < len(out_groups):
                        out_groups[out_pos[0]]()
                        out_pos[0] += 1
                    else:
                        break

            def consume_due(slot_idx):
                while fill_pos[0] < len(filler) and \
                        filler[fill_pos[0]][0] <= slot_idx:
                    filler[fill_pos[0]][1]()
                    fill_pos[0] += 1

            def consume_outproj(n):
                a = out_pos[0]
                b = min(a + n, len(out_groups))
                for i in range(a, b):
                    out_groups[i]()
                out_pos[0] = b

            # upfront: all of v, and q/k for (hp=0, tt=0..1); later q/k
            # tiles stream in as attention-slot filler.
            for kc in range(NKB // 2):
                v_chunk(kc)()
            for tt in range(2):
                for unit in (qk_halves(wqs, qT[0], 0, tt),
                             qk_halves(wks, kT[0], 0, tt)):
                    for h in unit:
                        h()

            # ---------------- attention ----------------
            # each slot's normalization is deferred into the next slot so
            # the ~3us DVE->PE->DVE chain runs while the PE streams the next
            # slot's scores; o_psum rotation (bufs=2) then unblocks in time.
            pending_norm = [None]

            def attention_slot(hp, qb):
                consume_due(4 * hp + qb)
                diag = [4 * qb + i for i in range(4)]
                order = diag + list(range(4 * qb))
                n_kb = len(order)
                o_ps = [o_psum.tile([P, TQ], F32, tag="o", name="o")
                        for _ in range(2)]
                s_tiles = {}
                pts = {}

                def c0_of(kb):
                    return max(0, kb * P - qb * TQ)

                def issue_scores(kb):
                    c0 = c0_of(kb)
                    sp = s_psum.tile([P, 2, TQ], F32, tag="s", name="s")
                    for j in range(2):
                        nc.tensor.matmul(
                            sp[:, j, c0:TQ],
                            kT[hp][j * 64:(j + 1) * 64, kb * P:(kb + 1) * P],
                            qT[hp][j * 64:(j + 1) * 64,
                                   qb * TQ + c0:(qb + 1) * TQ],
                            tile_position=(j * 64, 0))
                    s_tiles[kb] = (sp, c0)

                def issue_exp(kb):
                    sp, c0 = s_tiles.pop(kb)
                    pt = p_pool.tile([P, 2, TQ], F16, tag="p", name="p")
                    nc.scalar.activation(pt[:, :, c0:TQ], sp[:, :, c0:TQ],
                                         AF.Exp, scale=0.125)
                    if kb >= 4 * qb:
                        # triangle window of the diagonal tile
                        nc.gpsimd.affine_select(
                            out=pt[:, :, c0:c0 + P], in_=pt[:, :, c0:c0 + P],
                            pattern=[[0, 2], [1, P]],
                            compare_op=mybir.AluOpType.is_ge,
                            fill=0.0, base=0, channel_multiplier=-1)
                    pts[kb] = (pt, c0)

                def issue_pv(kb, first, last):
                    pt, c0 = pts.pop(kb)
                    for j in range(2):
                        nc.tensor.matmul(
                            o_ps[j][0:65, c0:TQ],
                            vA[kb][:, 2 * hp + j, :],
                            pt[:, j, c0:TQ],
                            start=first, stop=last)

                issue_scores(order[0])
                issue_scores(order[1])
                if pending_norm[0] is not None:
                    consume_filler(1)
                    pending_norm[0]()
                    pending_norm[0] = None
                issue_exp(order[0])
                consume_filler(1)
                for i, kb in enumerate(order):
                    if i + 2 < n_kb:
                        issue_scores(order[i + 2])
                    if i + 1 < n_kb:
                        issue_exp(order[i + 1])
                    issue_pv(kb, first=(i == 0), last=(i == n_kb - 1))
                    if i % 4 == 3:
                        consume_filler(1)

                # normalization: rows 0..63 of o_ps = unnormalized out.T,
                # row 64 = sum(exp).  Sums copied out, then their
                # reciprocal is broadcast across 64 partitions via a K=1
                # ones matmul into the unused upper half of the o_ps bank,
                # and the multiply writes aoT.
                def norm():
                    for j in range(2):
                        sc = r_pool.tile([1, TQ], F32R, tag="sc", name="sc")
                        nc.vector.tensor_copy(sc[:], o_ps[j][64:65, :])
                        rb = mm_psum.tile([P, TQ], F32, tag="mm", name="rb")
                        nc.tensor.matmul(rb[0:64, :], ones1[:], sc[:],
                                         start=True, stop=True)
                        R = r_pool.tile([64, TQ], F32, tag="Rb", name="R")
                        nc.vector.reciprocal_approx_fast(R[:], rb[0:64, :])
                        nc.vector.tensor_mul(
                            aoT[hp][j * 64:(j + 1) * 64,
                                    qb * TQ:(qb + 1) * TQ],
                            o_ps[j][0:64, :], R[:])
                pending_norm[0] = norm

            # out-projection groups for tt become available once hp=3's
            # norm for qb=tt has been issued (inside the NEXT slot, since
            # norms are deferred).
            for hp in range(NHP):
                for qb in range(NQB):
                    attention_slot(hp, qb)
                    if hp == 3 and qb >= 1:
                        out_groups.extend(
                            outproj_group(od, qb - 1) for od in range(ND))
                        consume_outproj(8)
            pending_norm[0]()
            pending_norm[0] = None
            out_groups.extend(outproj_group(od, 3) for od in range(ND))
            consume_filler(len(filler) - fill_pos[0])
            consume_outproj(len(out_groups))

    nc.compile()
    return nc


def _get_compiled(precision=None):
    key = precision or PRECISION
    if key not in _COMPILED:
        _COMPILED[key] = _build(key)
    return _COMPILED[key]


def make_in_maps(x, w_q, w_k, w_v, w_o, precision=None):
    xTs = [np.ascontiguousarray(x[b].T).astype(np.float16) for b in range(B)]
    wq = [np.ascontiguousarray(w_q[h * 512:(h + 1) * 512].T).astype(np.float16)
          for h in range(2)]
    wk = [np.ascontiguousarray(w_k[h * 512:(h + 1) * 512].T).astype(np.float16)
          for h in range(2)]
    wv = [np.ascontiguousarray(w_v[h * 512:(h + 1) * 512].T).astype(np.float16)
          for h in range(2)]
    wo = [np.ascontiguousarray(w_o[:, h * 512:(h + 1) * 512].T).astype(np.float16)
          for h in range(2)]
    in_maps = []
    for c in range(8):
        b, half = divmod(c, 2)
        in_maps.append({
            "xT": xTs[b],
            "wqT": wq[half],
            "wkT": wk[half],
            "wvT": wv[half],
            "woT": wo[half],
        })
    return in_maps


def kernel(x, w_q, w_k, w_v, w_o):
    from concourse.bass_utils import run_bass_kernel_spmd

    x = np.asarray(x, dtype=np.float32)
    w_q = np.asarray(w_q, dtype=np.float32)
    w_k = np.asarray(w_k, dtype=np.float32)
    w_v = np.asarray(w_v, dtype=np.float32)
    w_o = np.asarray(w_o, dtype=np.float32)

    nc = _get_compiled()
    in_maps = make_in_maps(x, w_q, w_k, w_v, w_o)
    res = run_bass_kernel_spmd(nc, in_maps, list(range(8)))

    out = np.empty((B, T, D), dtype=np.float32)
    for b in range(B):
        out[b] = (res.results[2 * b]["poutT"] + res.results[2 * b + 1]["poutT"]).T
    return out
